# revision 46
# baseline (speedup 1.0000x reference)
"""Trainium2 Bass kernel for nn_Attention_33354716021131.

Dense GQA attention block (B=2, S=2048, D=4096, 32 q-heads / 8 kv-heads,
head_dim 128, RoPE, causal softmax) tensor-parallel across 8 NeuronCores.

Sharding (per core c):
  - heads: q-heads 4c..4c+3 (one kv-head group c) -> wq/wk/wv column shards
  - x is transposed on the HOST and the full x^T [D, T] (bf16) is fed to
    every core -- no transpose phase, no x AllGather on device.
  - attention entirely local to the core (its 4 q-heads x 2 batches)
  - attention outputs (head-major, transposed, bf16) AllGather -> full O^T,
    then wo column shard: core c computes y[:, 512c:512c+512]; host
    concatenates.

Phase order is chosen so the two AllGathers overlap compute that does NOT
touch HBM (attention is SBUF-resident):
    qkv(b0) -> qkv(b1) -> attn(b0) -> AG(b0) || attn(b1) -> AG(b1) || wo(b0)
    -> wo(b1)
The Pool engine is used ONLY for the collectives (normalization broadcast
runs as a rank-1 PE matmul), so attn(b1) never queues behind AG(b0)'s
completion wait.

Layouts: everything stays in "transposed" [feature, token] form until wo.
  - QKV:   qT/kT/vT tile = w_tile.T @ xT_tile       (bf16 x bf16, acc fp32)
  - RoPE:  pair-swap via a constant permutation matmul, cos/sin on DVE
  - S^T:   sT[k,q] = kT_tile.T @ qT_tile            (bf16)
  - P^T:   exp on ScalarE (scale fused, bf16 out), tri-mask on diagonal
  - PV:    oT[d,q] += v_nat_tile.T @ pT_tile        (bf16)
  - den:   DVE accumulates pT; ones-matmul reduces partitions; reciprocal
           broadcast back via rank-1 matmul + ScalarE copy
  - WO:    y[tok, cols] = oT_tile.T @ wo_tile       (bf16 x bf16)
"""
import math
import os

import numpy as np
import ml_dtypes

N_CORES = 8
B = 2
S = 2048
DM = 4096
N_HEADS = 32
HD = 128
NQH = N_HEADS // N_CORES          # 4 q heads per core
HDQ = NQH * HD                    # 512
T = B * S                         # 4096 tokens
KC = DM // 128                    # 32 k-chunks
TB = 512                          # token block for projections
NTB = S // TB                     # 4 per batch
QB = 512                          # query block for attention
NQB = S // QB                     # 4
NKT = S // 128                    # 16 key tiles per batch
TG = 256                          # token group for the wo projection
SCALE = 1.0 / math.sqrt(HD)
ROPE_THETA = 10000.0

_CACHE = {}


def _consts():
    i = np.arange(HD // 2)
    inv = 1.0 / (ROPE_THETA ** (2 * i / HD))
    t = np.arange(S)
    ang = np.outer(inv, t)  # [64, S]
    cosT = np.repeat(np.cos(ang), 2, axis=0).astype(np.float32)  # [128, S]
    sinT = np.repeat(np.sin(ang), 2, axis=0).astype(np.float32)
    perm = np.zeros((128, 128), np.float32)
    for j in range(64):
        perm[2 * j, 2 * j + 1] = 1.0
        perm[2 * j + 1, 2 * j] = -1.0
    tri = (np.arange(128)[:, None] <= np.arange(128)[None, :]).astype(np.float32)
    ident = np.eye(128, dtype=np.float32)
    ones = np.ones((128, 1), np.float32)
    ones_row = np.ones((1, 128), np.float32)
    return cosT, sinT, perm, tri, ident, ones, ones_row


def _build(sim=False):
    import concourse.mybir as mybir
    import concourse.tile as tile
    from concourse import bacc

    F32 = mybir.dt.float32
    F32R = mybir.dt.float32r
    BF16 = mybir.dt.bfloat16

    nc = bacc.Bacc("TRN2", target_bir_lowering=False, debug=False,
                   num_devices=N_CORES)

    xT = nc.dram_tensor("xT", [DM, T], BF16, kind="ExternalInput")
    wq = nc.dram_tensor("wq", [DM, HDQ], BF16, kind="ExternalInput")
    wk = nc.dram_tensor("wk", [DM, HD], BF16, kind="ExternalInput")
    wv = nc.dram_tensor("wv", [DM, HD], BF16, kind="ExternalInput")
    wo = nc.dram_tensor("wo", [DM, HDQ], BF16, kind="ExternalInput")
    cosc = nc.dram_tensor("cosc", [128, S], BF16, kind="ExternalInput")
    sinc = nc.dram_tensor("sinc", [128, S], BF16, kind="ExternalInput")
    permc = nc.dram_tensor("permc", [128, 128], F32, kind="ExternalInput")
    tric = nc.dram_tensor("tric", [128, 128], F32, kind="ExternalInput")
    identc = nc.dram_tensor("identc", [128, 128], F32, kind="ExternalInput")
    onesc = nc.dram_tensor("onesc", [128, 1], F32, kind="ExternalInput")
    onesr = nc.dram_tensor("onesr", [1, 128], BF16, kind="ExternalInput")

    y = nc.dram_tensor("y", [T, HDQ], F32, kind="ExternalOutput")

    rg = [list(range(N_CORES))]

    with tile.TileContext(nc) as tc:
        with (
            tc.tile_pool(name="dram", bufs=1, space="DRAM") as dram,
            tc.tile_pool(name="const", bufs=1) as cp,
        ):
            cos_sb = cp.tile([128, S], BF16, tag="cos")
            sin_sb = cp.tile([128, S], BF16, tag="sin")
            perm_sb = cp.tile([128, 128], F32R, tag="perm")
            tri_sb = cp.tile([128, 128], F32, tag="tri")
            tri_bf = cp.tile([128, 128], BF16, tag="tri_bf")
            id_sb = cp.tile([128, 128], F32, tag="id")
            ones_sb = cp.tile([128, 1], F32R, tag="ones")
            onesr_sb = cp.tile([1, 128], BF16, tag="onesr")

            def load_consts_small():
                nc.sync.dma_start(out=tri_sb[:], in_=tric.ap())
                nc.sync.dma_start(out=id_sb[:], in_=identc.ap())
                nc.sync.dma_start(out=perm_sb[:], in_=permc.ap().bitcast(F32R))
                nc.sync.dma_start(out=ones_sb[:], in_=onesc.ap().bitcast(F32R))
                nc.sync.dma_start(out=onesr_sb[:], in_=onesr.ap())
                nc.vector.tensor_scalar_add(tri_bf[:], tri_sb[:], 0.0)

            def load_consts_big():
                nc.sync.dma_start(out=cos_sb[:], in_=cosc.ap())
                nc.sync.dma_start(out=sin_sb[:], in_=sinc.ap())

            oT_h = [dram.tile([HDQ, S], BF16, name=f"oT_h{b}") for b in range(B)]
            oT_F = [dram.tile([DM, S], BF16, addr_space="Shared",
                              name=f"oT_F{b}") for b in range(B)]

            # ---------- weights (resident across both batches) ----------
            with tc.tile_pool(name="wqkv", bufs=1) as wpool:
                wq_sb = wpool.tile([128, KC * HDQ], BF16, tag="wq")
                wk_sb = wpool.tile([128, KC * HD], BF16, tag="wk")
                wv_sb = wpool.tile([128, KC * HD], BF16, tag="wv")
                def load_kv_half(hf):
                    ksl = slice(hf * 16, (hf + 1) * 16)
                    nc.sync.dma_start(
                        out=wk_sb[:].rearrange(
                            "p (kc d) -> p kc d", kc=KC)[:, ksl],
                        in_=wk.ap().rearrange(
                            "(kc p) d -> p kc d", p=128)[:, ksl],
                    )
                    nc.sync.dma_start(
                        out=wv_sb[:].rearrange(
                            "p (kc d) -> p kc d", kc=KC)[:, ksl],
                        in_=wv.ap().rearrange(
                            "(kc p) d -> p kc d", p=128)[:, ksl],
                    )

                def load_wq_piece(kq):
                    ksl4 = slice(kq * 4, (kq + 1) * 4)
                    nc.sync.dma_start(
                        out=wq_sb[:].rearrange(
                            "p (kc d) -> p kc d", kc=KC)[:, ksl4],
                        in_=wq.ap().rearrange(
                            "(kc p) d -> p kc d", p=128)[:, ksl4],
                    )
                load_kv_half(0)
                load_wq_piece(0)
                hooks = {("kc", 0, 1): lambda: load_wq_piece(1),
                         ("kc", 0, 4): lambda: load_wq_piece(2),
                         ("kc", 0, 7): lambda: (load_wq_piece(3),
                                                load_kv_half(1)),
                         ("kc", 0, 11): lambda: load_wq_piece(4),
                         ("kc", 0, 15): lambda: load_wq_piece(5),
                         ("kc", 0, 19): lambda: (load_wq_piece(6),
                                                 load_wq_piece(7)),
                         ("kc", 0, 10): load_consts_small,
                         ("kc", 0, 24): load_consts_big}

                with (
                    tc.tile_pool(name="batch", bufs=2) as bp,
                    tc.tile_pool(name="wo_p", bufs=1) as wo_p,
                    tc.tile_pool(name="otp", bufs=2) as otp,
                ):
                    wo_sb = wo_p.tile([128, KC * HDQ], BF16, tag="wo")
                    qTb, kTb, vb = [], [], []
                    for b in range(B):
                        qT = [bp.tile([128, S], BF16, tag=f"qT{h}",
                                      name=f"qT{h}")
                              for h in range(NQH)]
                        kT = bp.tile([128, S], BF16, tag="kT")
                        v_nat = bp.tile([128, NKT * 128], BF16, tag="v_nat")
                        qTb.append(qT)
                        kTb.append(kT)
                        vb.append(v_nat)
                        _qkv_phase(nc, tc, mybir, b, xT, wq_sb, wk_sb, wv_sb,
                                   cos_sb, sin_sb, perm_sb, id_sb,
                                   qT, kT, v_nat, F32, F32R,
                                   hooks if b == 0 else None)

                    # wo prefetch: HBM is free while attention runs
                    nc.sync.dma_start(
                        out=wo_sb[:].rearrange("p (kc d) -> p kc d", kc=KC),
                        in_=wo.ap().rearrange("(kc p) d -> p kc d", p=128),
                    )

                    pre_strips = {}
                    for b in range(B):
                        _attn_phase(nc, tc, mybir, b, qTb[b], kTb[b], vb[b],
                                    tri_bf, ones_sb, onesr_sb, oT_h[b],
                                    F32, F32R, BF16)
                        if not sim:
                            nc.gpsimd.collective_compute(
                                "AllGather", mybir.AluOpType.bypass,
                                replica_groups=rg,
                                ins=[oT_h[b][:].opt()],
                                outs=[oT_F[b][:].opt()],
                            )
                        if b == 0:
                            # first strips of batch 0 on the Pool queue:
                            # it idles right after the AG(b0) wait, so the
                            # transfers start the moment the gather lands
                            # and never block the sync queue
                            for tg in range(2):
                                strip = otp.tile([128, KC * TG], BF16,
                                                 tag="strip", name="strip")
                                nc.gpsimd.dma_start(
                                    out=strip[:].rearrange(
                                        "p (hc t) -> p hc t", hc=KC),
                                    in_=oT_F[0][:]
                                    .rearrange("(hc p) t -> p hc t", p=128)
                                    [:, :, tg * TG:(tg + 1) * TG],
                                )
                                pre_strips[(0, tg)] = strip

                    # ---------- WO projection (column shard) ----------
                    with (
                        tc.tile_pool(name="ps_y", bufs=2,
                                     space="PSUM") as ps_y,
                        tc.tile_pool(name="w_wo", bufs=2) as wp,
                    ):
                        for b in range(B):
                            for tg in range(S // TG):
                                strip = pre_strips.pop((b, tg), None)
                                if strip is None:
                                    strip = otp.tile([128, KC * TG], BF16,
                                                     tag="strip",
                                                     name="strip")
                                    nc.sync.dma_start(
                                        out=strip[:].rearrange(
                                            "p (hc t) -> p hc t", hc=KC),
                                        in_=oT_F[b][:]
                                        .rearrange("(hc p) t -> p hc t",
                                                   p=128)
                                        [:, :, tg * TG:(tg + 1) * TG],
                                    )
                                for sub in range(TG // 128):
                                    psy = ps_y.tile([128, HDQ], F32,
                                                    tag="psy")
                                    for hc in range(KC):
                                        nc.tensor.matmul(
                                            psy[:],
                                            strip[:, hc * TG + sub * 128:
                                                  hc * TG + (sub + 1) * 128],
                                            wo_sb[:, hc * HDQ:
                                                  (hc + 1) * HDQ],
                                            start=(hc == 0),
                                            stop=(hc == KC - 1),
                                        )
                                    y_sb = wp.tile([128, HDQ], F32,
                                                   tag="y_sb")
                                    nc.scalar.copy(y_sb[:], psy[:])
                                    row = b * S + tg * TG + sub * 128
                                    nc.sync.dma_start(
                                        out=y.ap()[row:row + 128, :],
                                        in_=y_sb[:])

    nc.compile()
    return nc


def _qkv_phase(nc, tc, mybir, b, xT, wq_sb, wk_sb, wv_sb, cos_sb, sin_sb,
               perm_sb, id_sb, qT, kT, v_nat, F32, F32R, hooks=None):
    with (
        tc.tile_pool(name=f"ps_acc{b}", bufs=1, space="PSUM") as ps_acc,
        tc.tile_pool(name=f"ps_rope{b}", bufs=1, space="PSUM") as ps_rope,
        tc.tile_pool(name=f"wqk{b}", bufs=2) as wp,
        tc.tile_pool(name=f"xtp{b}", bufs=4) as xtp,
    ):
        BF16 = mybir.dt.bfloat16
        for tb in range(NTB):
            tsl = slice(tb * TB, (tb + 1) * TB)
            psq = [ps_acc.tile([128, TB], F32, tag=f"psq{i}", name=f"psq{i}")
                   for i in range(NQH)]
            psk = ps_acc.tile([128, TB], F32, tag="psk")
            psv = ps_acc.tile([128, TB], F32, tag="psv")
            for kc in range(KC):
                xt_t = xtp.tile([128, TB], BF16, tag="xt_t")
                col = b * S + tb * TB
                nc.sync.dma_start(
                    out=xt_t[:],
                    in_=xT.ap()[kc * 128:(kc + 1) * 128, col:col + TB],
                )
                if hooks and ("kc", tb, kc) in hooks:
                    hooks[("kc", tb, kc)]()
                for i in range(NQH):
                    nc.tensor.matmul(
                        psq[i][:],
                        wq_sb[:, kc * HDQ + i * HD:kc * HDQ + (i + 1) * HD],
                        xt_t[:],
                        start=(kc == 0), stop=(kc == KC - 1),
                    )
                nc.tensor.matmul(
                    psk[:], wk_sb[:, kc * HD:(kc + 1) * HD], xt_t[:],
                    start=(kc == 0), stop=(kc == KC - 1),
                )
                nc.tensor.matmul(
                    psv[:], wv_sb[:, kc * HD:(kc + 1) * HD], xt_t[:],
                    start=(kc == 0), stop=(kc == KC - 1),
                )

            if hooks and ("pre_rope", tb) in hooks:
                hooks[("pre_rope", tb)]()
            cos_t = cos_sb[:, tsl]
            sin_t = sin_sb[:, tsl]
            for idx in range(NQH + 1):
                acc = psq[idx] if idx < NQH else psk
                dest = qT[idx][:] if idx < NQH else kT[:]
                raw = wp.tile([128, TB], F32R, tag="rope_raw")
                nc.scalar.copy(raw[:], acc[:])
                swp = ps_rope.tile([128, TB], F32, tag="swp")
                nc.tensor.matmul(swp[:], perm_sb[:], raw[:],
                                 start=True, stop=True)
                t1 = wp.tile([128, TB], F32, tag="rope_t1")
                nc.vector.tensor_mul(t1[:], raw[:].bitcast(F32), cos_t)
                t2 = wp.tile([128, TB], F32, tag="rope_t2")
                nc.vector.tensor_mul(t2[:], swp[:], sin_t)
                nc.vector.tensor_add(dest[:, tsl], t1[:], t2[:])

            vt_sb = wp.tile([128, TB], F32, tag="vt_sb")
            nc.scalar.copy(vt_sb[:], psv[:])
            vp = ps_rope.tile([128, TB], F32, tag="vp")
            for j in range(TB // 128):
                nc.tensor.transpose(
                    vp[:, j * 128:(j + 1) * 128],
                    vt_sb[:, j * 128:(j + 1) * 128], id_sb[:])
            nc.scalar.copy(v_nat[:, tb * TB:(tb + 1) * TB], vp[:])


def _attn_phase(nc, tc, mybir, b, qT, kT, v_nat, tri_bf, ones_sb, onesr_sb,
                oT_hb, F32, F32R, BF16):
    with (
        tc.tile_pool(name=f"ps_s{b}", bufs=3, space="PSUM") as ps_s,
        tc.tile_pool(name=f"ps_o{b}", bufs=2, space="PSUM") as ps_o,
        tc.tile_pool(name=f"ps_den{b}", bufs=1, space="PSUM") as ps_den,
        tc.tile_pool(name=f"wa{b}", bufs=2) as wp,
        tc.tile_pool(name=f"ptp{b}", bufs=4) as ptp,
        tc.tile_pool(name=f"accp{b}", bufs=1) as accp,
    ):
        for h in range(NQH):
            for qb in range(NQB):
                q0 = qb * QB
                kt_max = (q0 + QB) // 128 - 1
                oT = ps_o.tile([128, QB], F32, tag="oT")
                acc = accp.tile([128, QB], F32R, tag="acc")
                for kt in range(kt_max + 1):
                    off = max(0, kt * 128 - q0)
                    qs = slice(q0 + off, q0 + QB)
                    psl = slice(off, QB)
                    sT = ps_s.tile([128, QB], F32, tag="sT")
                    pT = ptp.tile([128, QB], BF16, tag="pT")
                    nc.tensor.matmul(
                        sT[:, psl],
                        kT[:, kt * 128:(kt + 1) * 128],
                        qT[h][:, qs],
                        start=True, stop=True,
                    )
                    nc.scalar.activation(
                        pT[:, psl], sT[:, psl],
                        mybir.ActivationFunctionType.Exp,
                        scale=SCALE,
                    )
                    if kt * 128 >= q0:
                        nc.vector.tensor_mul(
                            pT[:, off:off + 128], pT[:, off:off + 128],
                            tri_bf[:])
                    nc.tensor.matmul(
                        oT[:, psl],
                        v_nat[:, kt * 128:(kt + 1) * 128],
                        pT[:, psl],
                        start=(kt == 0), stop=(kt == kt_max),
                    )
                    if kt == 0:
                        nc.vector.tensor_scalar_add(acc[:], pT[:], 0.0)
                    else:
                        nc.vector.tensor_add(
                            acc[:, psl],
                            acc[:, psl].bitcast(F32),
                            pT[:, psl])
                den = ps_den.tile([1, QB], F32, tag="den")
                nc.tensor.matmul(den[0:1, :], ones_sb[:], acc[:],
                                 start=True, stop=True)
                den_sb = wp.tile([1, QB], F32, tag="den_sb")
                nc.scalar.copy(den_sb[:], den[0:1, :])
                rec = wp.tile([1, QB], F32, tag="rec")
                scr = wp.tile([1, QB], F32, tag="scr")
                nc.vector.reciprocal_approx_accurate(
                    rec[:], den_sb[:], scr[:])
                recb = wp.tile([1, QB], BF16, tag="recb")
                nc.vector.tensor_scalar_add(recb[:], rec[:], 0.0)
                rbp = ps_den.tile([128, QB], F32, tag="rbp")
                nc.tensor.matmul(rbp[:], onesr_sb[:], recb[:],
                                 start=True, stop=True)
                rb = wp.tile([128, QB], F32, tag="rb")
                nc.scalar.copy(rb[:], rbp[:])
                oT_sb = wp.tile([128, QB], BF16, tag="oT_sb")
                nc.vector.tensor_mul(oT_sb[:], oT[:], rb[:])
                nc.sync.dma_start(
                    out=oT_hb[:][h * 128:(h + 1) * 128, q0:q0 + QB],
                    in_=oT_sb[:],
                )


def _in_maps(x, wq, wk, wv, wo):
    bf16 = ml_dtypes.bfloat16
    x2 = np.asarray(x, dtype=np.float32).reshape(T, DM)
    xT = np.ascontiguousarray(x2.T).astype(bf16)
    cosT, sinT, perm, tri, ident, ones, ones_row = _consts()
    maps = []
    for c in range(N_CORES):
        qsl = slice(c * HDQ, (c + 1) * HDQ)
        ksl = slice(c * HD, (c + 1) * HD)
        maps.append({
            "xT": xT,
            "wq": np.ascontiguousarray(np.asarray(wq)[:, qsl]).astype(bf16),
            "wk": np.ascontiguousarray(np.asarray(wk)[:, ksl]).astype(bf16),
            "wv": np.ascontiguousarray(np.asarray(wv)[:, ksl]).astype(bf16),
            "wo": np.ascontiguousarray(np.asarray(wo)[:, qsl]).astype(bf16),
            "cosc": cosT.astype(bf16), "sinc": sinT.astype(bf16),
            "permc": perm, "tric": tri,
            "identc": ident, "onesc": ones,
            "onesr": ones_row.astype(bf16),
        })
    return maps


def kernel(x, wq, wk, wv, wo, start_pos=0, **_unused):
    from concourse import bass_utils

    assert int(np.asarray(start_pos)) == 0
    in_maps = _in_maps(x, wq, wk, wv, wo)

    if "nc" not in _CACHE:
        _CACHE["nc"] = _build()
    nc = _CACHE["nc"]

    res = bass_utils.run_bass_kernel_spmd(
        nc, in_maps, core_ids=list(range(N_CORES)),
        trace=bool(int(os.environ.get("KERNEL_TRACE", "0") or 0)),
    )
    _CACHE["last_result"] = res

    out = np.empty((T, DM), np.float32)
    for c in range(N_CORES):
        out[:, c * HDQ:(c + 1) * HDQ] = res.results[c]["y"]
    return out.reshape(B, S, DM)
